# revision 1
# baseline (speedup 1.0000x reference)
"""Trainium2 Bass kernel for nn_Dihedral (gnn_message_passing, 8 NeuronCores).

kernel(**inputs) -> [256] f32 per-batch dihedral energies.

Design: mapping columns are consecutive-atom windows (b..b+3), so every
per-dihedral quantity except the batch label is a function of the window
start.  Host prep builds a 256 B per-window record table
    [pos[w..w+3] (12 f32), -k*cos(th) (3), -k*sin(th) (3), sum_k (1), pad]
(256 B = the dma_gather granularity), shards windows by atom range across the
8 cores (4 int16-addressable subtables each, per the shard-the-dihedral-dim
hint, with the small theta/k tables folded into the records), and routes each
dihedral as an int16 record index to the core owning its window, ordered by
batch with bins padded to 32-element blocks.

Device per core: tiles of 128*F dihedrals; dma_gather (4 SWDGE queues,
<=8192 idxs/call) pulls records into SBUF; DVE/ACT compute the torsion
(x = n1.n2, y = -(a.n2)*|b|; cos = x/r, sin = y/r, Chebyshev for d=2,3 --
no atan/cos tables needed); PE reduces each 32-element block of V via a
[128,4] indicator matmul.  Host: bincount block sums into the 256 bins and
sum the 8 per-core partials (the all-reduce of the sum-sharded output).
"""

import sys
import numpy as np

if "/opt/trn_rl_repo" not in sys.path:
    sys.path.insert(0, "/opt/trn_rl_repo")

import concourse.bass as bass
import concourse.bacc as bacc
import concourse.mybir as mybir
import concourse.tile as tile
from concourse.library_config import mlp
from concourse.tile_rust import add_dep_helper

P = 128
ELEM = 64            # f32 slots per record = 256B
NCORES = 8
QUANT = 32           # bin padding quantum == PE group size
NGRP = P // QUANT    # 4 partial sums per column


# --------------------------------------------------------------------------
# host-side prep
# --------------------------------------------------------------------------

def build_record_table(pos, atom_types, thetas, ks):
    """[NW, 19] f32 window records (not yet sharded/padded)."""
    NW = pos.shape[0] - 3
    t3 = thetas.reshape(3, -1).astype(np.float64)
    k3 = ks.reshape(3, -1).astype(np.float64)
    A = (-(k3 * np.cos(t3))).astype(np.float32)      # [3, 390625]
    B = (-(k3 * np.sin(t3))).astype(np.float32)
    C = k3.sum(axis=0).astype(np.float32)            # [390625]
    ty = np.asarray(atom_types).astype(np.int64)
    T4 = ((ty[:NW] * 25 + ty[1:NW + 1]) * 25 + ty[2:NW + 2]) * 25 + ty[3:NW + 3]
    rec = np.empty((NW, 19), dtype=np.float32)
    rec[:, 0:3] = pos[0:NW]
    rec[:, 3:6] = pos[1:NW + 1]
    rec[:, 6:9] = pos[2:NW + 2]
    rec[:, 9:12] = pos[3:NW + 3]
    rec[:, 12] = A[0, T4]; rec[:, 13] = A[1, T4]; rec[:, 14] = A[2, T4]
    rec[:, 15] = B[0, T4]; rec[:, 16] = B[1, T4]; rec[:, 17] = B[2, T4]
    rec[:, 18] = C[T4]
    return rec


def plan_layout(base, batch, n_win, F, gcall=64):
    """Shard dihedrals by window range; per (core, subtable) order by batch,
    pad bins to QUANT and groups to P, sizes common across cores.

    Returns dict with per-core index arrays, call plan, and block labels."""
    NSUB = 4
    SUBT = (n_win + NCORES * NSUB - 1) // (NCORES * NSUB)   # rows per subtable
    assert SUBT <= 32000, SUBT
    DUMMY = SUBT                                            # dummy record slot
    SUBT_ALLOC = ((SUBT + 1 + 63) // 64) * 64               # dram rows per sub

    sub_g = base // SUBT                 # global subtable 0..31
    local = (base - sub_g * SUBT).astype(np.int32)
    core_of = sub_g // NSUB
    sub_l = sub_g % NSUB

    # per (core, sub): element lists (already batch-sorted since input order is)
    per = {}
    for c in range(NCORES):
        for s in range(NSUB):
            sel = np.nonzero((core_of == c) & (sub_l == s))[0]
            per[(c, s)] = sel

    # per (core, sub): pad each batch-bin to a QUANT multiple
    padded_idx = {}      # int16 record index stream (DUMMY for padding)
    padded_lab = {}      # per-QUANT-block batch label
    for (c, s), sel in per.items():
        lab = batch[sel]
        cnt = np.bincount(lab, minlength=256)
        pcnt = ((cnt + QUANT - 1) // QUANT) * QUANT
        total = int(pcnt.sum())
        idx_out = np.full(total, DUMMY, dtype=np.int16)
        ends = np.cumsum(pcnt)
        starts = ends - pcnt
        # positions of real elements: starts[lab] + rank within bin
        within = np.arange(len(sel)) - np.repeat(np.cumsum(cnt) - cnt, cnt)
        pos_out = starts[lab] + within
        idx_out[pos_out] = local[sel].astype(np.int16)
        blk_lab = np.repeat(np.arange(256), pcnt // QUANT)
        padded_idx[(c, s)] = idx_out
        padded_lab[(c, s)] = blk_lab

    # common per-sub column counts across cores (pad with full-dummy cols)
    ncols_sub = []
    for s in range(4):
        m = max(len(padded_idx[(c, s)]) for c in range(NCORES))
        ncols_sub.append((m + P - 1) // P)

    # gather-call plan: (sub, cols<=gcall) chunks; compute groups pack
    # consecutive calls up to F columns
    calls = []
    for s in range(4):
        rem = ncols_sub[s]
        while rem > 0:
            f = min(gcall, rem)
            calls.append((s, f))
            rem -= f
    ncols_tot = sum(f for _, f in calls)
    groups = []
    cur = []
    cur_cols = 0
    for (s, f) in calls:
        if cur and cur_cols + f > F:
            groups.append(cur)
            cur, cur_cols = [], 0
        cur.append((s, f))
        cur_cols += f
    if cur:
        groups.append(cur)

    # per-core wrapped idx stream + block labels
    idx_dram = np.zeros((NCORES, P, 8 * ncols_tot), dtype=np.int16)
    blk_labels = np.full((NCORES, ncols_tot * NGRP), -1, dtype=np.int32)
    for c in range(NCORES):
        col0 = 0
        for s in range(4):
            arr = padded_idx[(c, s)]
            full = np.full(ncols_sub[s] * P, DUMMY, dtype=np.int16)
            full[:len(arr)] = arr
            lab = padded_lab[(c, s)]
            labfull = np.full(ncols_sub[s] * NGRP, -1, dtype=np.int32)
            labfull[:len(lab)] = lab
            blk_labels[c, col0 * NGRP:(col0 + ncols_sub[s]) * NGRP] = labfull
            # wrapped layout per call
            cc = col0
            off = 0
            for (ss, f) in calls:
                if ss != s:
                    continue
                chunk = full[off:off + f * P]
                if len(chunk) < f * P:
                    break
                wrapped = chunk.reshape(8 * f, 16).T          # [16, 8f]
                idx_dram[c, :, cc * 8:(cc + f) * 8] = np.tile(wrapped, (8, 1))
                off += f * P
                cc += f
            col0 += ncols_sub[s]
    return dict(SUBT=SUBT, SUBT_ALLOC=SUBT_ALLOC, DUMMY=DUMMY, calls=calls,
                groups=groups, ncols_tot=ncols_tot, idx_dram=idx_dram,
                blk_labels=blk_labels, ncols_sub=ncols_sub)


def build_core_tables(rec, plan):
    """[NCORES, 4*SUBT_ALLOC, ELEM] f32 sharded record tables."""
    SUBT, SUBT_ALLOC, DUMMY = plan["SUBT"], plan["SUBT_ALLOC"], plan["DUMMY"]
    NW = rec.shape[0]
    tables = np.zeros((NCORES, 4 * SUBT_ALLOC, ELEM), dtype=np.float32)
    # dummy record: valid geometry (window 0), zero coefficients
    dummy_rec = np.zeros(ELEM, dtype=np.float32)
    dummy_rec[:12] = rec[0, :12]
    for c in range(NCORES):
        for s in range(4):
            g = c * 4 + s
            lo = g * SUBT
            hi = min(lo + SUBT, NW)
            if hi > lo:
                tables[c, s * SUBT_ALLOC: s * SUBT_ALLOC + (hi - lo), :19] = rec[lo:hi]
            tables[c, s * SUBT_ALLOC + DUMMY] = dummy_rec
    return tables


# --------------------------------------------------------------------------
# device program
# --------------------------------------------------------------------------

def build_program(plan, repeat=1, mode="full", nqueues=1):
    SUBT_ALLOC = plan["SUBT_ALLOC"]
    groups = plan["groups"]
    ncols_tot = plan["ncols_tot"]
    f32 = mybir.dt.float32
    Alu = mybir.AluOpType
    Act = mybir.ActivationFunctionType

    nc = bacc.Bacc("TRN2", target_bir_lowering=False, debug=False,
                   num_swdge_queues=nqueues)
    tbl = nc.dram_tensor("tbl", [4 * SUBT_ALLOC, ELEM], f32, kind="ExternalInput").ap()
    idxs = nc.dram_tensor("idxs", [P, 8 * ncols_tot], mybir.dt.int16,
                          kind="ExternalInput").ap()
    out = nc.dram_tensor("out", [NGRP, ncols_tot], f32, kind="ExternalOutput").ap()

    with tile.TileContext(nc) as tc:
        with (
            tc.tile_pool(name="gat", bufs=3) as gat_pool,
            tc.tile_pool(name="tmp", bufs=1) as tmp_pool,
            tc.tile_pool(name="cst", bufs=1) as cst_pool,
            tc.tile_pool(name="ps", bufs=2, space="PSUM") as ps_pool,
        ):
            lib_inst = nc.gpsimd.load_library(mlp)

            grp = cst_pool.tile([P, NGRP], f32)
            nc.gpsimd.memset(grp[:], 0.0)
            for g in range(NGRP):
                nc.gpsimd.memset(grp[g * QUANT:(g + 1) * QUANT, g:g + 1], 1.0)

            bs = cst_pool.tile([NGRP, ncols_tot], f32)

            first_gather = [None]
            call_no = [0]

            def do_group(group, col0):
                F_j = sum(f for _, f in group)
                g = gat_pool.tile([P, F_j * ELEM], f32, tag="g")
                off = 0
                for (sub, f) in group:
                    if mode == "compute":
                        off += f
                        continue
                    N = P * f
                    it = tmp_pool.tile([P, 8 * f], mybir.dt.int16, tag="idx",
                                       name="idx", bufs=8)
                    nc.sync.dma_start(
                        out=it[:],
                        in_=idxs[:, (col0 + off) * 8:(col0 + off + f) * 8])
                    gi = nc.gpsimd.dma_gather(
                        g[:, off * ELEM:(off + f) * ELEM].rearrange(
                            "p (f e) -> p f e", e=ELEM),
                        tbl[sub * SUBT_ALLOC:(sub + 1) * SUBT_ALLOC, :],
                        it[:], N, N, ELEM, single_packet=False,
                        queue_num=call_no[0] % nqueues,
                    )
                    call_no[0] += 1
                    if first_gather[0] is None:
                        first_gather[0] = gi
                        add_dep_helper(lib_inst.ins, gi.ins, sync=False,
                                       reason="library before gather")
                    off += f
                if mode == "compute":
                    nc.vector.memset(g[:], 1.0)

                ge = g[:].rearrange("p (f e) -> p e f", e=ELEM)

                def fld(k):
                    return ge[:, k, :]

                if mode == "gather":
                    pt0 = ps_pool.tile([NGRP, F_j], f32, tag="ps", name="ps")
                    nc.tensor.matmul(out=pt0[:], lhsT=grp[:], rhs=fld(0),
                                     start=True, stop=True)
                    nc.scalar.activation(bs[:, col0:col0 + F_j], pt0[:], Act.Copy)
                    return

                def T(tag):
                    return tmp_pool.tile([P, F_j], f32, tag=tag, name=tag)

                def tt(o, i0, i1, op):
                    nc.vector.tensor_tensor(out=o, in0=i0, in1=i1, op=op)

                # dr vectors (strided reads from the gathered records)
                abc = []
                for j, (hi, lo) in enumerate([(3, 0), (6, 3), (9, 6)]):
                    for k in range(3):
                        t = T(f"d{j}{k}")
                        tt(t[:], fld(hi + k), fld(lo + k), Alu.subtract)
                        abc.append(t)
                ax, ay, az, bx, by, bz, cx, cy, cz = abc

                def cross(ux, uy, uz, vx, vy, vz, tag):
                    o = []
                    for k, (m1, m2, m3, m4) in enumerate([
                            (uy, vz, uz, vy), (uz, vx, ux, vz), (ux, vy, uy, vx)]):
                        t1 = T(f"{tag}t{k}")
                        t2 = T(f"{tag}u{k}")
                        tt(t1[:], m1[:], m2[:], Alu.mult)
                        tt(t2[:], m3[:], m4[:], Alu.mult)
                        tt(t1[:], t1[:], t2[:], Alu.subtract)
                        o.append(t1)
                    return o

                n1 = cross(ax, ay, az, bx, by, bz, "n1")
                n2 = cross(bx, by, bz, cx, cy, cz, "n2")

                def dot(u, v, tag):
                    acc = T(tag)
                    t = T(tag + "t")
                    tt(acc[:], u[0][:], v[0][:], Alu.mult)
                    tt(t[:], u[1][:], v[1][:], Alu.mult)
                    tt(acc[:], acc[:], t[:], Alu.add)
                    tt(t[:], u[2][:], v[2][:], Alu.mult)
                    tt(acc[:], acc[:], t[:], Alu.add)
                    return acc

                x = dot(n1, n2, "x")
                D = dot([ax, ay, az], n2, "D")
                w = dot([bx, by, bz], [bx, by, bz], "w")
                L = T("L"); nc.scalar.sqrt(L[:], w[:])
                p = T("p"); tt(p[:], D[:], L[:], Alu.mult)
                qa = T("qa"); nc.scalar.square(qa[:], x[:])
                qb = T("qb"); nc.scalar.square(qb[:], p[:])
                tt(qa[:], qa[:], qb[:], Alu.add)
                r = T("r"); nc.scalar.sqrt(r[:], qa[:])
                inv = T("inv"); nc.vector.reciprocal_approx_fast(inv[:], r[:])
                c1 = T("c1"); tt(c1[:], x[:], inv[:], Alu.mult)
                s1 = T("s1")
                nc.vector.scalar_tensor_tensor(
                    out=s1[:], in0=p[:], scalar=-1.0, in1=inv[:],
                    op0=Alu.mult, op1=Alu.mult)
                cc = T("cc"); nc.scalar.square(cc[:], c1[:])
                c2 = T("c2")
                nc.scalar.activation(c2[:], cc[:], Act.Copy, bias=-1.0, scale=2.0)
                s2 = T("s2")
                nc.vector.scalar_tensor_tensor(
                    out=s2[:], in0=s1[:], scalar=2.0, in1=c1[:],
                    op0=Alu.mult, op1=Alu.mult)
                r3 = T("r3")
                nc.scalar.activation(r3[:], cc[:], Act.Copy, bias=-3.0, scale=4.0)
                c3 = T("c3"); tt(c3[:], r3[:], c1[:], Alu.mult)
                r4 = T("r4")
                nc.scalar.activation(r4[:], cc[:], Act.Copy, bias=-1.0, scale=4.0)
                s3 = T("s3"); tt(s3[:], r4[:], s1[:], Alu.mult)

                V = T("V"); t = T("Vt")
                tt(V[:], fld(12), c1[:], Alu.mult)          # A1*c1
                tt(V[:], V[:], fld(18), Alu.add)            # + C
                for fk, cs in [(15, s1), (13, c2), (16, s2), (14, c3), (17, s3)]:
                    tt(t[:], fld(fk), cs[:], Alu.mult)
                    tt(V[:], V[:], t[:], Alu.add)

                pt = ps_pool.tile([NGRP, F_j], f32, tag="ps")
                nc.tensor.matmul(out=pt[:], lhsT=grp[:], rhs=V[:],
                                 start=True, stop=True)
                nc.scalar.activation(bs[:, col0:col0 + F_j], pt[:], Act.Copy)

            def body():
                col0 = 0
                for grp_calls in groups:
                    do_group(grp_calls, col0)
                    col0 += sum(f for _, f in grp_calls)

            if repeat > 1:
                with tc.For_i(0, repeat, 1):
                    body()
            else:
                body()

            nc.sync.dma_start(out=out[:], in_=bs[:])
    nc.compile()
    return nc


# --------------------------------------------------------------------------
# end to end
# --------------------------------------------------------------------------

def prepare(inputs, F=192, gcall=64):
    pos = np.asarray(inputs["pos"], dtype=np.float32)
    ty = np.asarray(inputs["atom_types"])
    mapping = np.asarray(inputs["mapping"])
    batch = np.asarray(inputs["mapping_batch"]).astype(np.int64)
    base = np.asarray(mapping[0]).astype(np.int64)
    assert all(np.array_equal(np.asarray(mapping[j]), base + j) for j in range(1, 4)), \
        "mapping not consecutive; fast path invalid"
    n_win = pos.shape[0] - 3
    rec = build_record_table(pos, ty, np.asarray(inputs["thetas"]),
                             np.asarray(inputs["ks"]))
    plan = plan_layout(base, batch, n_win, F, gcall=gcall)
    tables = build_core_tables(rec, plan)
    return plan, tables


def finish(plan, outs, n_batch=256):
    """outs: list per core of [NGRP, ncols_tot] block sums -> [256] energy."""
    energy = np.zeros(n_batch, dtype=np.float64)
    for c in range(NCORES):
        bsums = np.asarray(outs[c])          # [NGRP, ncols]
        lab = plan["blk_labels"][c]          # [ncols*NGRP], -1 = padding
        vals = bsums.T.ravel()               # block (col,grp) order
        m = lab >= 0
        energy += np.bincount(lab[m], weights=vals[m].astype(np.float64),
                              minlength=n_batch)
    return energy.astype(np.float32)


def _kernel_numpy_fallback(pos, atom_types, mapping, mapping_batch, thetas, ks):
    # Correctness safety net for non-consecutive mappings (never expected).
    p0, p1 = pos[mapping[0]], pos[mapping[1]]
    p2, p3 = pos[mapping[2]], pos[mapping[3]]
    dr1, dr2, dr3 = p1 - p0, p2 - p1, p3 - p2
    n1 = np.cross(dr1, dr2); n2 = np.cross(dr2, dr3)
    m1 = np.cross(n1, dr2 / np.linalg.norm(dr2, axis=-1, keepdims=True))
    x = np.sum(n1 * n2, -1); y = np.sum(m1 * n2, -1)
    theta = np.arctan2(y, x)
    t0, t1, t2, t3 = (atom_types[mapping[j]] for j in range(4))
    th = thetas[:, t0, t1, t2, t3]; kk = ks[:, t0, t1, t2, t3]
    degs = np.arange(1, 4)[:, None]
    V = np.sum(kk * (1.0 - np.cos(degs * theta[None, :] - th)), axis=0)
    return np.bincount(mapping_batch, weights=V.astype(np.float64),
                       minlength=256).astype(np.float32)


def kernel(pos, atom_types, mapping, mapping_batch, thetas, ks):
    from concourse.bass_utils import run_bass_kernel_spmd
    pos = np.asarray(pos, dtype=np.float32)
    atom_types = np.asarray(atom_types)
    mapping = np.asarray(mapping)
    mapping_batch = np.asarray(mapping_batch)
    thetas = np.asarray(thetas, dtype=np.float32)
    ks = np.asarray(ks, dtype=np.float32)

    base = np.asarray(mapping[0]).astype(np.int64)
    if not all(np.array_equal(np.asarray(mapping[j]), base + j)
               for j in range(1, 4)):
        print("kernel.py: non-consecutive mapping; numpy fallback",
              file=sys.stderr)
        return _kernel_numpy_fallback(pos, atom_types, mapping, mapping_batch,
                                      thetas, ks)

    batch = mapping_batch.astype(np.int64)
    n_win = pos.shape[0] - 3
    rec = build_record_table(pos, atom_types, thetas, ks)
    plan = plan_layout(base, batch, n_win, F=176, gcall=16)
    tables = build_core_tables(rec, plan)

    nc = build_program(plan, repeat=1, mode="full", nqueues=4)
    in_maps = [{"tbl": tables[c], "idxs": plan["idx_dram"][c]}
               for c in range(NCORES)]
    res = run_bass_kernel_spmd(nc, in_maps, list(range(NCORES)))
    outs = [res.results[c]["out"] for c in range(NCORES)]
    return finish(plan, outs).astype(np.float32)



# revision 5
# speedup vs baseline: 7.9954x; 7.9954x over previous
"""Trainium2 Bass kernel for nn_Dihedral (gnn_message_passing, 8 NeuronCores).

kernel(**inputs) -> [256] f32 per-batch dihedral energies.

Design v2 — gather-free streaming. mapping columns are consecutive-atom
windows (b..b+3), so every per-dihedral quantity except the batch label is a
function of the window start. The host builds, per core, a batch-sorted
per-dihedral stream of 16 bf16 field planes
    [dr1 (3), dr2 (3), dr3 (3), A1..A3, B1'..B3', C]
(A = -k*cos(th0), B' = sign/2x-folded -k*sin(th0), C = sum_k — the same
type-table fold as v1) laid out tile-contiguously in DRAM, so the device does
pure SEQUENTIAL DMA (no dma_gather, no 256B-record padding: 32B/dihedral vs
256B in v1). Bins are padded to 32-element blocks along partitions.

Device per core, per [128, F] tile: one dma_start pulls all 16 planes;
DVE (bf16 2x_1p) + ACT + gpsimd compute the torsion via the xy-polynomial
form (cos d*phi, sin d*phi expanded in x = n1.n2, Y = (dr1.n2)|dr2|, u = 1/r
— no trig tables), PE reduces 32-partition blocks with a [128,4] indicator
matmul. Host: bincount block sums into 256 bins, sum the 8 per-core partials
(the all-reduce of the sum-sharded output).
"""

import sys
import numpy as np

if "/opt/trn_rl_repo" not in sys.path:
    sys.path.insert(0, "/opt/trn_rl_repo")

import concourse.bass as bass
import concourse.bacc as bacc
import concourse.mybir as mybir
import concourse.tile as tile
from concourse.library_config import standard as std_lib
from concourse.tile_rust import add_dep_helper
import ml_dtypes

P = 128
NCORES = 8
QUANT = 32           # bin padding quantum == PE group size
NGRP = P // QUANT    # 4 partial sums per column
NPLANES = 16
NB = 256


# --------------------------------------------------------------------------
# host-side prep
# --------------------------------------------------------------------------

def build_window_fields(pos, atom_types, thetas, ks):
    """[16, NW] f32 per-window field planes."""
    NW = pos.shape[0] - 3
    t3 = thetas.reshape(3, -1).astype(np.float64)
    k3 = ks.reshape(3, -1).astype(np.float64)
    A = (-(k3 * np.cos(t3))).astype(np.float32)      # [3, 390625]
    B = (-(k3 * np.sin(t3))).astype(np.float32)
    C = k3.sum(axis=0).astype(np.float32)
    ty = np.asarray(atom_types).astype(np.int64)
    T4 = ((ty[:NW] * 25 + ty[1:NW + 1]) * 25 + ty[2:NW + 2]) * 25 + ty[3:NW + 3]
    f = np.empty((NPLANES, NW), dtype=np.float32)
    f[0:3] = (pos[1:NW + 1] - pos[0:NW]).T
    f[3:6] = (pos[2:NW + 2] - pos[1:NW + 1]).T
    f[6:9] = (pos[3:NW + 3] - pos[2:NW + 2]).T
    f[9] = A[0, T4]; f[10] = A[1, T4]; f[11] = A[2, T4]
    f[12] = -B[0, T4]; f[13] = -2.0 * B[1, T4]; f[14] = -B[2, T4]
    f[15] = C[T4]
    return f


def plan_streams(base, batch, n_win, F):
    """Per-core batch-sorted window-index streams, bins padded to QUANT,
    common NTILES across cores. Returns (streams [NCORES, NCOLS*P] int64
    with -1 dummies, blk_labels [NCORES, NCOLS*NGRP] int32, NTILES)."""
    SUBW = (n_win + NCORES - 1) // NCORES
    core_of = base // SUBW

    idx_streams = []
    lab_streams = []
    for c in range(NCORES):
        sel = np.nonzero(core_of == c)[0]        # batch-sorted already
        lab = batch[sel]
        cnt = np.bincount(lab, minlength=NB)
        pcnt = ((cnt + QUANT - 1) // QUANT) * QUANT
        total = int(pcnt.sum())
        idx_out = np.full(total, -1, dtype=np.int64)
        ends = np.cumsum(pcnt)
        starts = ends - pcnt
        within = np.arange(len(sel)) - np.repeat(np.cumsum(cnt) - cnt, cnt)
        idx_out[starts[lab] + within] = base[sel]
        blk_lab = np.repeat(np.arange(NB, dtype=np.int32), pcnt // QUANT)
        idx_streams.append(idx_out)
        lab_streams.append(blk_lab)

    max_cols = max((len(s) + P - 1) // P for s in idx_streams)
    NTILES = (max_cols + F - 1) // F
    NCOLS = NTILES * F

    streams = np.full((NCORES, NCOLS * P), -1, dtype=np.int64)
    blk_labels = np.full((NCORES, NCOLS * NGRP), -1, dtype=np.int32)
    for c in range(NCORES):
        streams[c, :len(idx_streams[c])] = idx_streams[c]
        blk_labels[c, :len(lab_streams[c])] = lab_streams[c]
    return streams, blk_labels, NTILES


def build_core_tables(fields, streams, NTILES, F, np_dtype):
    """[NCORES, NTILES*P, NPLANES*F] plane tables (tile-contiguous DRAM)."""
    NW = fields.shape[1]
    tables = np.empty((NCORES, NTILES * P, NPLANES * F), dtype=np_dtype)
    for c in range(NCORES):
        w = streams[c]
        dummy = w < 0
        vals = fields[:, np.where(dummy, 0, w)]          # [16, NCOLS*P]
        if dummy.any():
            vals[9:16][:, dummy] = 0.0                   # V = 0 for padding
        # stream s = t*(P*F) + col*P + p  ->  dram[t*P + p, k*F + col]
        v4 = vals.reshape(NPLANES, NTILES, F, P)
        tables[c] = (v4.transpose(1, 3, 0, 2)
                     .reshape(NTILES * P, NPLANES * F).astype(np_dtype))
    return tables


# --------------------------------------------------------------------------
# device program
# --------------------------------------------------------------------------

def build_program(NTILES, F, repeat=1, dtype="bf16"):
    f32 = mybir.dt.float32
    dt = mybir.dt.bfloat16 if dtype == "bf16" else f32
    Alu = mybir.AluOpType
    Act = mybir.ActivationFunctionType
    NCOLS = NTILES * F

    nc = bacc.Bacc("TRN2", target_bir_lowering=False, debug=False)
    tbl = nc.dram_tensor("tbl", [NTILES * P, NPLANES * F], dt,
                         kind="ExternalInput").ap()
    out = nc.dram_tensor("out", [NGRP, NCOLS], f32, kind="ExternalOutput").ap()

    with tile.TileContext(nc) as tc:
        with (
            tc.tile_pool(name="gat", bufs=3) as gat_pool,
            tc.tile_pool(name="tmp", bufs=2) as tmp_pool,
            tc.tile_pool(name="cst", bufs=1) as cst_pool,
            tc.tile_pool(name="ps", bufs=2, space="PSUM") as ps_pool,
        ):
            lib_inst = nc.gpsimd.load_library(std_lib)

            grp = cst_pool.tile([P, NGRP], dt)
            nc.gpsimd.memset(grp[:], 0.0)
            for g in range(NGRP):
                nc.gpsimd.memset(grp[g * QUANT:(g + 1) * QUANT, g:g + 1], 1.0)

            bs = cst_pool.tile([NGRP, NCOLS], f32)

            first_pool_op = [None]

            def dve_tt(o, i0, i1, op):
                nc.vector.tensor_tensor(out=o[:], in0=i0, in1=i1, op=op)

            def pool_tt(o, i0, i1, op):
                i = nc.gpsimd.tensor_tensor(out=o[:], in0=i0, in1=i1, op=op)
                if first_pool_op[0] is None:
                    first_pool_op[0] = i
                    add_dep_helper(lib_inst.ins, i.ins, sync=False,
                                   reason="library before pool ops")

            def do_tile(t):
                g = gat_pool.tile([P, NPLANES * F], dt, tag="g")
                nc.sync.dma_start(out=g[:], in_=tbl[t * P:(t + 1) * P, :])

                def fld(k):
                    return g[:, k * F:(k + 1) * F]

                def T(tag, d=dt):
                    return tmp_pool.tile([P, F], d, tag=tag, name=tag)

                ax, ay, az = fld(0), fld(1), fld(2)
                bx, by, bz = fld(3), fld(4), fld(5)
                cx, cy, cz = fld(6), fld(7), fld(8)

                # n1 = dr1 x dr2, n2 = dr2 x dr3                     [DVE]
                def cross(ux, uy, uz, vx, vy, vz, tag):
                    o = []
                    for k, (m1, m2, m3, m4) in enumerate([
                            (uy, vz, uz, vy), (uz, vx, ux, vz), (ux, vy, uy, vx)]):
                        t1 = T(f"{tag}t{k}")
                        t2 = T(f"{tag}u{k}")
                        dve_tt(t1, m1, m2, Alu.mult)
                        dve_tt(t2, m3, m4, Alu.mult)
                        dve_tt(t1, t1[:], t2[:], Alu.subtract)
                        o.append(t1[:])
                    return o

                n1 = cross(ax, ay, az, bx, by, bz, "n1")
                n2 = cross(bx, by, bz, cx, cy, cz, "n2")

                def dot(u, v, tag):
                    acc = T(tag)
                    tt = T(tag + "t")
                    dve_tt(acc, u[0], v[0], Alu.mult)
                    dve_tt(tt, u[1], v[1], Alu.mult)
                    dve_tt(acc, acc[:], tt[:], Alu.add)
                    dve_tt(tt, u[2], v[2], Alu.mult)
                    dve_tt(acc, acc[:], tt[:], Alu.add)
                    return acc

                x = dot(n1, n2, "x")[:]                             # n1.n2
                D = dot([ax, ay, az], n2, "D")[:]                   # dr1.n2

                # w = dr2.dr2: squares on ACT, adds on Pool
                bxx = T("bxx"); nc.scalar.square(bxx[:], bx)
                byy = T("byy"); nc.scalar.square(byy[:], by)
                bzz = T("bzz"); nc.scalar.square(bzz[:], bz)
                w1 = T("w1"); pool_tt(w1, bxx[:], byy[:], Alu.add)
                w = T("w"); pool_tt(w, w1[:], bzz[:], Alu.add)
                L = T("L"); nc.scalar.sqrt(L[:], w[:])

                Y = T("Y"); dve_tt(Y, D, L[:], Alu.mult)            # (dr1.n2)|dr2|

                xx = T("xx"); nc.scalar.square(xx[:], x)
                YY = T("YY"); nc.scalar.square(YY[:], Y[:])
                q = T("q"); pool_tt(q, xx[:], YY[:], Alu.add)
                r = T("r", f32); nc.scalar.sqrt(r[:], q[:])
                uf = T("uf", f32)
                nc.vector.reciprocal_approx_fast(uf[:], r[:])
                u = T("u"); nc.scalar.copy(u[:], uf[:])
                u2 = T("u2"); nc.scalar.square(u2[:], u[:])
                u3 = T("u3"); dve_tt(u3, u2[:], u[:], Alu.mult)

                xY = T("xY"); dve_tt(xY, x, Y[:], Alu.mult)
                xmy = T("xmy"); dve_tt(xmy, xx[:], YY[:], Alu.subtract)

                # c3a = xx - 3YY = xmy - 2YY; s3a = 3xx - YY = xmy + 2xx
                YY2 = T("YY2"); nc.scalar.mul(YY2[:], YY[:], 2.0)
                xx2 = T("xx2"); nc.scalar.mul(xx2[:], xx[:], 2.0)
                c3a = T("c3a"); pool_tt(c3a, xmy[:], YY2[:], Alu.subtract)
                s3a = T("s3a"); pool_tt(s3a, xmy[:], xx2[:], Alu.add)
                c3 = T("c3"); dve_tt(c3, c3a[:], x, Alu.mult)
                s3 = T("s3"); dve_tt(s3, s3a[:], Y[:], Alu.mult)

                # P_d = A_d * cos-part + B'_d * sin-part; G_d = P_d * u^d
                p1 = T("p1"); t1 = T("t1")
                dve_tt(p1, fld(9), x, Alu.mult)
                dve_tt(t1, fld(12), Y[:], Alu.mult)
                pool_tt(p1, p1[:], t1[:], Alu.add)
                g1 = T("g1"); dve_tt(g1, p1[:], u[:], Alu.mult)

                p2 = T("p2"); t2 = T("t2")
                dve_tt(p2, fld(10), xmy[:], Alu.mult)
                dve_tt(t2, fld(13), xY[:], Alu.mult)
                pool_tt(p2, p2[:], t2[:], Alu.add)
                g2 = T("g2"); dve_tt(g2, p2[:], u2[:], Alu.mult)

                p3 = T("p3"); t3 = T("t3")
                dve_tt(p3, fld(11), c3[:], Alu.mult)
                dve_tt(t3, fld(14), s3[:], Alu.mult)
                pool_tt(p3, p3[:], t3[:], Alu.add)
                g3 = T("g3"); dve_tt(g3, p3[:], u3[:], Alu.mult)

                V = T("V")
                pool_tt(V, fld(15), g1[:], Alu.add)
                pool_tt(V, V[:], g2[:], Alu.add)
                dve_tt(V, V[:], g3[:], Alu.add)

                pt = ps_pool.tile([NGRP, F], f32, tag="ps", name="ps")
                nc.tensor.matmul(out=pt[:], lhsT=grp[:], rhs=V[:],
                                 start=True, stop=True)
                nc.scalar.activation(bs[:, t * F:(t + 1) * F], pt[:], Act.Copy)

            def body():
                for t in range(NTILES):
                    do_tile(t)

            if repeat > 1:
                with tc.For_i(0, repeat, 1):
                    body()
            else:
                body()

            nc.sync.dma_start(out=out[:], in_=bs[:])
    nc.compile()
    return nc


# --------------------------------------------------------------------------
# end to end
# --------------------------------------------------------------------------

def prepare(inputs, F=512, dtype="bf16"):
    pos = np.asarray(inputs["pos"], dtype=np.float32)
    ty = np.asarray(inputs["atom_types"])
    mapping = np.asarray(inputs["mapping"])
    batch = np.asarray(inputs["mapping_batch"]).astype(np.int64)
    base = np.asarray(mapping[0]).astype(np.int64)
    assert all(np.array_equal(np.asarray(mapping[j]), base + j)
               for j in range(1, 4)), "mapping not consecutive; fast path invalid"
    n_win = pos.shape[0] - 3
    fields = build_window_fields(pos, ty, np.asarray(inputs["thetas"]),
                                 np.asarray(inputs["ks"]))
    streams, blk_labels, NTILES = plan_streams(base, batch, n_win, F)
    np_dtype = ml_dtypes.bfloat16 if dtype == "bf16" else np.float32
    tables = build_core_tables(fields, streams, NTILES, F, np_dtype)
    plan = dict(NTILES=NTILES, F=F, blk_labels=blk_labels, dtype=dtype)
    return plan, tables


def finish(plan, outs, n_batch=NB):
    """outs: list per core of [NGRP, NCOLS] block sums -> [256] energy."""
    energy = np.zeros(n_batch, dtype=np.float64)
    for c in range(NCORES):
        bsums = np.asarray(outs[c])          # [NGRP, NCOLS]
        lab = plan["blk_labels"][c]          # [NCOLS*NGRP], -1 = padding
        vals = bsums.T.ravel()               # block (col, grp) order
        m = lab >= 0
        energy += np.bincount(lab[m], weights=vals[m].astype(np.float64),
                              minlength=n_batch)
    return energy.astype(np.float32)


def _kernel_numpy_fallback(pos, atom_types, mapping, mapping_batch, thetas, ks):
    # Correctness safety net for non-consecutive mappings (never expected).
    p0, p1 = pos[mapping[0]], pos[mapping[1]]
    p2, p3 = pos[mapping[2]], pos[mapping[3]]
    dr1, dr2, dr3 = p1 - p0, p2 - p1, p3 - p2
    n1 = np.cross(dr1, dr2); n2 = np.cross(dr2, dr3)
    m1 = np.cross(n1, dr2 / np.linalg.norm(dr2, axis=-1, keepdims=True))
    x = np.sum(n1 * n2, -1); y = np.sum(m1 * n2, -1)
    theta = np.arctan2(y, x)
    t0, t1, t2, t3 = (atom_types[mapping[j]] for j in range(4))
    th = thetas[:, t0, t1, t2, t3]; kk = ks[:, t0, t1, t2, t3]
    degs = np.arange(1, 4)[:, None]
    V = np.sum(kk * (1.0 - np.cos(degs * theta[None, :] - th)), axis=0)
    return np.bincount(mapping_batch, weights=V.astype(np.float64),
                       minlength=256).astype(np.float32)


def kernel(pos, atom_types, mapping, mapping_batch, thetas, ks):
    from concourse.bass_utils import run_bass_kernel_spmd
    pos = np.asarray(pos, dtype=np.float32)
    atom_types = np.asarray(atom_types)
    mapping = np.asarray(mapping)
    mapping_batch = np.asarray(mapping_batch)
    thetas = np.asarray(thetas, dtype=np.float32)
    ks = np.asarray(ks, dtype=np.float32)

    base = np.asarray(mapping[0]).astype(np.int64)
    if not all(np.array_equal(np.asarray(mapping[j]), base + j)
               for j in range(1, 4)):
        print("kernel.py: non-consecutive mapping; numpy fallback",
              file=sys.stderr)
        return _kernel_numpy_fallback(pos, atom_types, mapping, mapping_batch,
                                      thetas, ks)

    inputs = dict(pos=pos, atom_types=atom_types, mapping=mapping,
                  mapping_batch=mapping_batch, thetas=thetas, ks=ks)
    plan, tables = prepare(inputs, F=512, dtype="bf16")
    nc = build_program(plan["NTILES"], plan["F"], repeat=1,
                       dtype=plan["dtype"])
    in_maps = [{"tbl": tables[c]} for c in range(NCORES)]
    res = run_bass_kernel_spmd(nc, in_maps, list(range(NCORES)))
    outs = [res.results[c]["out"] for c in range(NCORES)]
    return finish(plan, outs).astype(np.float32)


# revision 10
# speedup vs baseline: 8.2585x; 1.0329x over previous
"""Trainium2 Bass kernel for nn_Dihedral (gnn_message_passing, 8 NeuronCores).

kernel(**inputs) -> [256] f32 per-batch dihedral energies.

Design v3 — gather-free streaming, engine-split compute. mapping columns are
consecutive-atom windows (b..b+3), so every per-dihedral quantity except the
batch label is a function of the window start. The host builds, per core, a
batch-sorted per-dihedral stream of 15 bf16 field planes
    [dr1 (3), dr2 (3), dr3 (3), A1..A3, B1'..B3']
(A = -k*cos(th0), B' = sign/2x-folded -k*sin(th0) — the same type-table fold
as v1; the angle-independent C = sum_k term is summed host-side) laid out
tile-contiguously in DRAM, so the device does pure SEQUENTIAL DMA (no
dma_gather, 30B/dihedral vs 256B in v1). Bins are padded to 32-element
blocks along partitions.

Device per core, per [128, F] tile: one dma_start pulls all 15 planes; the
columns are SPLIT between DVE (bf16 2x_1p) and gpsimd, each running the full
torsion chain independently on its slice (no cross-engine dependency
stalls); ACT does squares/sqrts/copies for both slices; PE reduces
32-partition blocks with a [128,4] indicator matmul. The torsion uses the
xy-polynomial form (cos/sin of d*phi expanded in x = n1.n2,
Y = (dr1.n2)|dr2|, u = 1/r — no trig tables). Host: bincount block sums
into 256 bins, sum the 8 per-core partials (the all-reduce of the
sum-sharded output).
"""

import os
import sys
import numpy as np

if "/opt/trn_rl_repo" not in sys.path:
    sys.path.insert(0, "/opt/trn_rl_repo")

import concourse.bass as bass
import concourse.bacc as bacc
import concourse.mybir as mybir
import concourse.tile as tile
from concourse.library_config import standard as std_lib
from concourse.tile_rust import add_dep_helper
import ml_dtypes

P = 128
NCORES = 8
QUANT = 32           # bin padding quantum == PE group size
NGRP = P // QUANT    # 4 partial sums per column
NPLANES = 15
NB = 256


# --------------------------------------------------------------------------
# host-side prep
# --------------------------------------------------------------------------

def build_window_fields(pos, atom_types, thetas, ks):
    """([15, NW] f32 per-window field planes, [NW] f32 C values)."""
    NW = pos.shape[0] - 3
    t3 = thetas.reshape(3, -1).astype(np.float64)
    k3 = ks.reshape(3, -1).astype(np.float64)
    A = (-(k3 * np.cos(t3))).astype(np.float32)      # [3, 390625]
    B = (-(k3 * np.sin(t3))).astype(np.float32)
    C = k3.sum(axis=0).astype(np.float32)
    ty = np.asarray(atom_types).astype(np.int64)
    T4 = ((ty[:NW] * 25 + ty[1:NW + 1]) * 25 + ty[2:NW + 2]) * 25 + ty[3:NW + 3]
    f = np.empty((NPLANES, NW), dtype=np.float32)
    f[0:3] = (pos[1:NW + 1] - pos[0:NW]).T
    f[3:6] = (pos[2:NW + 2] - pos[1:NW + 1]).T
    f[6:9] = (pos[3:NW + 3] - pos[2:NW + 2]).T
    f[9] = A[0, T4]; f[10] = A[1, T4]; f[11] = A[2, T4]
    f[12] = -B[0, T4]; f[13] = -2.0 * B[1, T4]; f[14] = -B[2, T4]
    return f, C[T4]


def plan_streams(base, batch, n_win, F):
    """Per-core batch-sorted window-index streams, bins padded to QUANT,
    common NTILES across cores."""
    SUBW = (n_win + NCORES - 1) // NCORES
    core_of = base // SUBW

    idx_streams = []
    lab_streams = []
    for c in range(NCORES):
        sel = np.nonzero(core_of == c)[0]        # batch-sorted already
        lab = batch[sel]
        cnt = np.bincount(lab, minlength=NB)
        pcnt = ((cnt + QUANT - 1) // QUANT) * QUANT
        total = int(pcnt.sum())
        idx_out = np.full(total, -1, dtype=np.int64)
        ends = np.cumsum(pcnt)
        starts = ends - pcnt
        within = np.arange(len(sel)) - np.repeat(np.cumsum(cnt) - cnt, cnt)
        idx_out[starts[lab] + within] = base[sel]
        blk_lab = np.repeat(np.arange(NB, dtype=np.int32), pcnt // QUANT)
        idx_streams.append(idx_out)
        lab_streams.append(blk_lab)

    max_cols = max((len(s) + P - 1) // P for s in idx_streams)
    NTILES = (max_cols + F - 1) // F
    NCOLS = NTILES * F

    streams = np.full((NCORES, NCOLS * P), -1, dtype=np.int64)
    blk_labels = np.full((NCORES, NCOLS * NGRP), -1, dtype=np.int32)
    for c in range(NCORES):
        streams[c, :len(idx_streams[c])] = idx_streams[c]
        blk_labels[c, :len(lab_streams[c])] = lab_streams[c]
    return streams, blk_labels, NTILES


def build_core_tables(fields, streams, NTILES, F, np_dtype):
    """[NCORES, NTILES*P, NPLANES*F] plane tables (tile-contiguous DRAM)."""
    tables = np.empty((NCORES, NTILES * P, NPLANES * F), dtype=np_dtype)
    for c in range(NCORES):
        w = streams[c]
        dummy = w < 0
        vals = fields[:, np.where(dummy, 0, w)]          # [15, NCOLS*P]
        if dummy.any():
            vals[9:15][:, dummy] = 0.0                   # V = 0 for padding
        # stream s = t*(P*F) + col*P + p  ->  dram[t*P + p, k*F + col]
        v4 = vals.reshape(NPLANES, NTILES, F, P)
        tables[c] = (v4.transpose(1, 3, 0, 2)
                     .reshape(NTILES * P, NPLANES * F).astype(np_dtype))
    return tables


# --------------------------------------------------------------------------
# device program
# --------------------------------------------------------------------------

def build_program(NTILES, F, repeat=1, dtype="bf16", split=None,
                  gat_bufs=None, tmp_bufs=None):
    if split is None:
        split = float(os.environ.get("DK_SPLIT", "1.0"))
    if gat_bufs is None:
        gat_bufs = int(os.environ.get("DK_GATB", "2"))
    if tmp_bufs is None:
        tmp_bufs = int(os.environ.get("DK_TMPB", "1"))
    f32 = mybir.dt.float32
    dt = mybir.dt.bfloat16 if dtype == "bf16" else f32
    Alu = mybir.AluOpType
    Act = mybir.ActivationFunctionType
    NCOLS = NTILES * F
    S = (int(F * split) // 4) * 4            # DVE columns; rest go to Pool

    nc = bacc.Bacc("TRN2", target_bir_lowering=False, debug=False)
    tbl = nc.dram_tensor("tbl", [NTILES * P, NPLANES * F], dt,
                         kind="ExternalInput").ap()
    out = nc.dram_tensor("out", [NGRP, NCOLS], f32, kind="ExternalOutput").ap()

    with tile.TileContext(nc) as tc:
        with (
            tc.tile_pool(name="gat", bufs=gat_bufs) as gat_pool,
            tc.tile_pool(name="tmp", bufs=tmp_bufs) as tmp_pool,
            tc.tile_pool(name="cst", bufs=1) as cst_pool,
            tc.tile_pool(name="ps", bufs=2, space="PSUM") as ps_pool,
        ):
            lib_inst = nc.gpsimd.load_library(std_lib)

            grp = cst_pool.tile([P, NGRP], dt)
            nc.gpsimd.memset(grp[:], 0.0)
            for g in range(NGRP):
                nc.gpsimd.memset(grp[g * QUANT:(g + 1) * QUANT, g:g + 1], 1.0)

            bs = cst_pool.tile([NGRP, NCOLS], f32)

            first_pool_op = [None]

            def do_side(g, V, side, lo, n):
                """Full torsion chain for columns [lo, lo+n) of tile g.
                side: 'd' -> tensor ops on DVE, 'p' -> on Pool."""
                def fld(k):
                    return g[:, k * F + lo:k * F + lo + n]

                def T(tag, d=dt):
                    return tmp_pool.tile([P, n], d, tag=f"{tag}_{side}",
                                         name=f"{tag}_{side}")

                if side == "d":
                    def tt(o, i0, i1, op):
                        nc.vector.tensor_tensor(out=o, in0=i0, in1=i1, op=op)
                else:
                    def tt(o, i0, i1, op):
                        i = nc.gpsimd.tensor_tensor(out=o, in0=i0, in1=i1,
                                                    op=op)
                        if first_pool_op[0] is None:
                            first_pool_op[0] = i
                            add_dep_helper(lib_inst.ins, i.ins, sync=False,
                                           reason="library before pool ops")

                ax, ay, az = fld(0), fld(1), fld(2)
                bx, by, bz = fld(3), fld(4), fld(5)
                cx, cy, cz = fld(6), fld(7), fld(8)

                cs = T("cs")
                def cross1(o, m1, m2, m3, m4):
                    tt(o[:], m1, m2, Alu.mult)
                    tt(cs[:], m3, m4, Alu.mult)
                    tt(o[:], o[:], cs[:], Alu.subtract)

                n1x = T("n1x"); cross1(n1x, ay, bz, az, by)
                n1y = T("n1y"); cross1(n1y, az, bx, ax, bz)
                n1z = T("n1z"); cross1(n1z, ax, by, ay, bx)
                n2x = T("n2x"); cross1(n2x, by, cz, bz, cy)
                n2y = T("n2y"); cross1(n2y, bz, cx, bx, cz)
                n2z = T("n2z"); cross1(n2z, bx, cy, by, cx)

                def dot3(o, u0, u1, u2_, v0, v1, v2):
                    st = T("dt")
                    tt(o[:], u0, v0, Alu.mult)
                    tt(st[:], u1, v1, Alu.mult)
                    tt(o[:], o[:], st[:], Alu.add)
                    tt(st[:], u2_, v2, Alu.mult)
                    tt(o[:], o[:], st[:], Alu.add)

                x = T("x"); dot3(x, n1x[:], n1y[:], n1z[:], n2x[:], n2y[:], n2z[:])
                D = T("D"); dot3(D, ax, ay, az, n2x[:], n2y[:], n2z[:])

                w = T("w"); wt = T("wt")
                nc.scalar.square(w[:], bx)
                nc.scalar.square(wt[:], by)
                tt(w[:], w[:], wt[:], Alu.add)
                nc.scalar.square(wt[:], bz)
                tt(w[:], w[:], wt[:], Alu.add)
                L = T("L"); nc.scalar.sqrt(L[:], w[:])
                Y = T("Y"); tt(Y[:], D[:], L[:], Alu.mult)

                xx = T("xx"); nc.scalar.square(xx[:], x[:])
                YY = T("YY"); nc.scalar.square(YY[:], Y[:])
                q = T("q"); tt(q[:], xx[:], YY[:], Alu.add)
                r = T("r", f32); nc.scalar.sqrt(r[:], q[:])
                uf = T("uf", f32)
                nc.vector.reciprocal_approx_fast(uf[:], r[:])
                u = T("u"); nc.scalar.copy(u[:], uf[:])
                u2 = T("u2"); nc.scalar.square(u2[:], u[:])
                u3 = T("u3"); tt(u3[:], u2[:], u[:], Alu.mult)

                xY = T("xY"); tt(xY[:], x[:], Y[:], Alu.mult)
                xmy = T("xmy"); tt(xmy[:], xx[:], YY[:], Alu.subtract)
                YY2 = T("YY2"); nc.scalar.mul(YY2[:], YY[:], 2.0)
                xx2 = T("xx2"); nc.scalar.mul(xx2[:], xx[:], 2.0)
                c3a = T("c3a"); tt(c3a[:], xmy[:], YY2[:], Alu.subtract)
                s3a = T("s3a"); tt(s3a[:], xmy[:], xx2[:], Alu.add)
                c3 = T("c3"); tt(c3[:], c3a[:], x[:], Alu.mult)
                s3 = T("s3"); tt(s3[:], s3a[:], Y[:], Alu.mult)

                pB = T("pB"); tB = T("tB"); gB = T("gB")
                Vs = V[:, lo:lo + n]
                # deg 1: V = (A1*x + B1'*Y)*u
                tt(pB[:], fld(9), x[:], Alu.mult)
                tt(tB[:], fld(12), Y[:], Alu.mult)
                tt(pB[:], pB[:], tB[:], Alu.add)
                tt(Vs, pB[:], u[:], Alu.mult)
                # deg 2: V += (A2*xmy + B2'*xY)*u2
                tt(pB[:], fld(10), xmy[:], Alu.mult)
                tt(tB[:], fld(13), xY[:], Alu.mult)
                tt(pB[:], pB[:], tB[:], Alu.add)
                tt(gB[:], pB[:], u2[:], Alu.mult)
                tt(Vs, Vs, gB[:], Alu.add)
                # deg 3: V += (A3*c3 + B3'*s3)*u3
                tt(pB[:], fld(11), c3[:], Alu.mult)
                tt(tB[:], fld(14), s3[:], Alu.mult)
                tt(pB[:], pB[:], tB[:], Alu.add)
                tt(gB[:], pB[:], u3[:], Alu.mult)
                tt(Vs, Vs, gB[:], Alu.add)

            def do_tile(t):
                g = gat_pool.tile([P, NPLANES * F], dt, tag="g")
                nc.sync.dma_start(out=g[:], in_=tbl[t * P:(t + 1) * P, :])
                V = tmp_pool.tile([P, F], dt, tag="V", name="V", bufs=2)
                do_side(g[:], V, "d", 0, S)
                if S < F:
                    do_side(g[:], V, "p", S, F - S)
                pt = ps_pool.tile([NGRP, F], f32, tag="ps", name="ps")
                for c0 in range(0, F, 512):
                    c1 = min(c0 + 512, F)
                    nc.tensor.matmul(out=pt[:, c0:c1], lhsT=grp[:],
                                     rhs=V[:, c0:c1], start=True, stop=True)
                nc.scalar.activation(bs[:, t * F:(t + 1) * F], pt[:], Act.Copy)

            def body():
                for t in range(NTILES):
                    do_tile(t)

            if repeat > 1:
                with tc.For_i(0, repeat, 1):
                    body()
            else:
                body()

            nc.sync.dma_start(out=out[:], in_=bs[:])
    nc.compile()
    return nc


# --------------------------------------------------------------------------
# end to end
# --------------------------------------------------------------------------

def prepare(inputs, F=1024, dtype="bf16"):
    pos = np.asarray(inputs["pos"], dtype=np.float32)
    ty = np.asarray(inputs["atom_types"])
    mapping = np.asarray(inputs["mapping"])
    batch = np.asarray(inputs["mapping_batch"]).astype(np.int64)
    base = np.asarray(mapping[0]).astype(np.int64)
    assert all(np.array_equal(np.asarray(mapping[j]), base + j)
               for j in range(1, 4)), "mapping not consecutive; fast path invalid"
    n_win = pos.shape[0] - 3
    fields, Cw = build_window_fields(pos, ty, np.asarray(inputs["thetas"]),
                                     np.asarray(inputs["ks"]))
    streams, blk_labels, NTILES = plan_streams(base, batch, n_win, F)
    np_dtype = ml_dtypes.bfloat16 if dtype == "bf16" else np.float32
    tables = build_core_tables(fields, streams, NTILES, F, np_dtype)
    # angle-independent sum_k term, accumulated host-side
    energy_C = np.bincount(batch, weights=Cw[base].astype(np.float64),
                           minlength=NB)
    plan = dict(NTILES=NTILES, F=F, blk_labels=blk_labels, dtype=dtype,
                energy_C=energy_C)
    return plan, tables


def finish(plan, outs, n_batch=NB):
    """outs: list per core of [NGRP, NCOLS] block sums -> [256] energy."""
    energy = plan["energy_C"].copy()
    for c in range(NCORES):
        bsums = np.asarray(outs[c])          # [NGRP, NCOLS]
        lab = plan["blk_labels"][c]          # [NCOLS*NGRP], -1 = padding
        vals = bsums.T.ravel()               # block (col, grp) order
        m = lab >= 0
        energy += np.bincount(lab[m], weights=vals[m].astype(np.float64),
                              minlength=n_batch)
    return energy.astype(np.float32)


def _kernel_numpy_fallback(pos, atom_types, mapping, mapping_batch, thetas, ks):
    # Correctness safety net for non-consecutive mappings (never expected).
    p0, p1 = pos[mapping[0]], pos[mapping[1]]
    p2, p3 = pos[mapping[2]], pos[mapping[3]]
    dr1, dr2, dr3 = p1 - p0, p2 - p1, p3 - p2
    n1 = np.cross(dr1, dr2); n2 = np.cross(dr2, dr3)
    m1 = np.cross(n1, dr2 / np.linalg.norm(dr2, axis=-1, keepdims=True))
    x = np.sum(n1 * n2, -1); y = np.sum(m1 * n2, -1)
    theta = np.arctan2(y, x)
    t0, t1, t2, t3 = (atom_types[mapping[j]] for j in range(4))
    th = thetas[:, t0, t1, t2, t3]; kk = ks[:, t0, t1, t2, t3]
    degs = np.arange(1, 4)[:, None]
    V = np.sum(kk * (1.0 - np.cos(degs * theta[None, :] - th)), axis=0)
    return np.bincount(mapping_batch, weights=V.astype(np.float64),
                       minlength=256).astype(np.float32)


def kernel(pos, atom_types, mapping, mapping_batch, thetas, ks):
    from concourse.bass_utils import run_bass_kernel_spmd
    pos = np.asarray(pos, dtype=np.float32)
    atom_types = np.asarray(atom_types)
    mapping = np.asarray(mapping)
    mapping_batch = np.asarray(mapping_batch)
    thetas = np.asarray(thetas, dtype=np.float32)
    ks = np.asarray(ks, dtype=np.float32)

    base = np.asarray(mapping[0]).astype(np.int64)
    if not all(np.array_equal(np.asarray(mapping[j]), base + j)
               for j in range(1, 4)):
        print("kernel.py: non-consecutive mapping; numpy fallback",
              file=sys.stderr)
        return _kernel_numpy_fallback(pos, atom_types, mapping, mapping_batch,
                                      thetas, ks)

    inputs = dict(pos=pos, atom_types=atom_types, mapping=mapping,
                  mapping_batch=mapping_batch, thetas=thetas, ks=ks)
    plan, tables = prepare(inputs, F=1024, dtype="bf16")
    nc = build_program(plan["NTILES"], plan["F"], repeat=1,
                       dtype=plan["dtype"])
    in_maps = [{"tbl": tables[c]} for c in range(NCORES)]
    res = run_bass_kernel_spmd(nc, in_maps, list(range(NCORES)))
    outs = [res.results[c]["out"] for c in range(NCORES)]
    return finish(plan, outs).astype(np.float32)


# revision 11
# speedup vs baseline: 12.1894x; 1.4760x over previous
"""Trainium2 Bass kernel for nn_Dihedral (gnn_message_passing, 8 NeuronCores).

kernel(**inputs) -> [256] f32 per-batch dihedral energies.

Design v3 — gather-free streaming, engine-split compute. mapping columns are
consecutive-atom windows (b..b+3), so every per-dihedral quantity except the
batch label is a function of the window start. The host builds, per core, a
batch-sorted per-dihedral stream of 15 bf16 field planes
    [dr1 (3), dr2 (3), dr3 (3), A1..A3, B1'..B3']
(A = -k*cos(th0), B' = sign/2x-folded -k*sin(th0) — the same type-table fold
as v1; the angle-independent C = sum_k term is summed host-side) laid out
tile-contiguously in DRAM, so the device does pure SEQUENTIAL DMA (no
dma_gather, 30B/dihedral vs 256B in v1). Bins are padded to 32-element
blocks along partitions.

Device per core, per [128, F] tile: one dma_start pulls all 15 planes; the
columns are SPLIT between DVE (bf16 2x_1p) and gpsimd, each running the full
torsion chain independently on its slice (no cross-engine dependency
stalls); ACT does squares/sqrts/copies for both slices; PE reduces
32-partition blocks with a [128,4] indicator matmul. The torsion uses the
xy-polynomial form (cos/sin of d*phi expanded in x = n1.n2,
Y = (dr1.n2)|dr2|, u = 1/r — no trig tables). Host: bincount block sums
into 256 bins, sum the 8 per-core partials (the all-reduce of the
sum-sharded output).
"""

import os
import sys
import numpy as np

if "/opt/trn_rl_repo" not in sys.path:
    sys.path.insert(0, "/opt/trn_rl_repo")

import concourse.bass as bass
import concourse.bacc as bacc
import concourse.mybir as mybir
import concourse.tile as tile
from concourse.library_config import standard as std_lib
from concourse.tile_rust import add_dep_helper
import ml_dtypes

P = 128
NCORES = 8
QUANT = 32           # bin padding quantum == PE group size
NGRP = P // QUANT    # 4 partial sums per column
NPLANES = 15
NB = 256


# --------------------------------------------------------------------------
# host-side prep
# --------------------------------------------------------------------------

def build_window_fields(pos, atom_types, thetas, ks):
    """([15, NW] f32 per-window field planes, [NW] f32 C values)."""
    NW = pos.shape[0] - 3
    t3 = thetas.reshape(3, -1).astype(np.float64)
    k3 = ks.reshape(3, -1).astype(np.float64)
    A = (-(k3 * np.cos(t3))).astype(np.float32)      # [3, 390625]
    B = (-(k3 * np.sin(t3))).astype(np.float32)
    C = k3.sum(axis=0).astype(np.float32)
    ty = np.asarray(atom_types).astype(np.int64)
    T4 = ((ty[:NW] * 25 + ty[1:NW + 1]) * 25 + ty[2:NW + 2]) * 25 + ty[3:NW + 3]
    f = np.empty((NPLANES, NW), dtype=np.float32)
    f[0:3] = (pos[1:NW + 1] - pos[0:NW]).T
    f[3:6] = (pos[2:NW + 2] - pos[1:NW + 1]).T
    f[6:9] = (pos[3:NW + 3] - pos[2:NW + 2]).T
    f[9] = A[0, T4]; f[10] = A[1, T4]; f[11] = A[2, T4]
    f[12] = -B[0, T4]; f[13] = -2.0 * B[1, T4]; f[14] = -B[2, T4]
    return f, C[T4]


def plan_streams(base, batch, n_win, F):
    """Per-core batch-sorted window-index streams, bins padded to QUANT,
    common NTILES across cores."""
    SUBW = (n_win + NCORES - 1) // NCORES
    core_of = base // SUBW

    idx_streams = []
    lab_streams = []
    for c in range(NCORES):
        sel = np.nonzero(core_of == c)[0]        # batch-sorted already
        lab = batch[sel]
        cnt = np.bincount(lab, minlength=NB)
        pcnt = ((cnt + QUANT - 1) // QUANT) * QUANT
        total = int(pcnt.sum())
        idx_out = np.full(total, -1, dtype=np.int64)
        ends = np.cumsum(pcnt)
        starts = ends - pcnt
        within = np.arange(len(sel)) - np.repeat(np.cumsum(cnt) - cnt, cnt)
        idx_out[starts[lab] + within] = base[sel]
        blk_lab = np.repeat(np.arange(NB, dtype=np.int32), pcnt // QUANT)
        idx_streams.append(idx_out)
        lab_streams.append(blk_lab)

    max_cols = max((len(s) + P - 1) // P for s in idx_streams)
    NTILES = (max_cols + F - 1) // F
    NCOLS = NTILES * F

    streams = np.full((NCORES, NCOLS * P), -1, dtype=np.int64)
    blk_labels = np.full((NCORES, NCOLS * NGRP), -1, dtype=np.int32)
    for c in range(NCORES):
        streams[c, :len(idx_streams[c])] = idx_streams[c]
        blk_labels[c, :len(lab_streams[c])] = lab_streams[c]
    return streams, blk_labels, NTILES


def build_core_tables(fields, streams, NTILES, F, np_dtype):
    """[NCORES, NTILES*P, NPLANES*F] plane tables (tile-contiguous DRAM)."""
    tables = np.empty((NCORES, NTILES * P, NPLANES * F), dtype=np_dtype)
    for c in range(NCORES):
        w = streams[c]
        dummy = w < 0
        vals = fields[:, np.where(dummy, 0, w)]          # [15, NCOLS*P]
        if dummy.any():
            vals[9:15][:, dummy] = 0.0                   # V = 0 for padding
        # stream s = t*(P*F) + col*P + p  ->  dram[t*P + p, k*F + col]
        v4 = vals.reshape(NPLANES, NTILES, F, P)
        tables[c] = (v4.transpose(1, 3, 0, 2)
                     .reshape(NTILES * P, NPLANES * F).astype(np_dtype))
    return tables


# --------------------------------------------------------------------------
# device program
# --------------------------------------------------------------------------

def build_program(NTILES, F, repeat=1, dtype="bf16", split=None,
                  gat_bufs=None, tmp_bufs=None):
    if split is None:
        split = float(os.environ.get("DK_SPLIT", "1.0"))
    if gat_bufs is None:
        gat_bufs = int(os.environ.get("DK_GATB", "2"))
    if tmp_bufs is None:
        tmp_bufs = int(os.environ.get("DK_TMPB", "1"))
    f32 = mybir.dt.float32
    dt = mybir.dt.bfloat16 if dtype == "bf16" else f32
    Alu = mybir.AluOpType
    Act = mybir.ActivationFunctionType
    NCOLS = NTILES * F
    S = (int(F * split) // 4) * 4            # DVE columns; rest go to Pool

    nc = bacc.Bacc("TRN2", target_bir_lowering=False, debug=False)
    tbl = nc.dram_tensor("tbl", [NTILES * P, NPLANES * F], dt,
                         kind="ExternalInput").ap()
    out = nc.dram_tensor("out", [NGRP, NCOLS], f32, kind="ExternalOutput").ap()

    with tile.TileContext(nc) as tc:
        with (
            tc.tile_pool(name="gat", bufs=gat_bufs) as gat_pool,
            tc.tile_pool(name="tmp", bufs=tmp_bufs) as tmp_pool,
            tc.tile_pool(name="cst", bufs=1) as cst_pool,
            tc.tile_pool(name="ps", bufs=2, space="PSUM") as ps_pool,
        ):
            lib_inst = nc.gpsimd.load_library(std_lib)

            grp = cst_pool.tile([P, NGRP], dt)
            nc.gpsimd.memset(grp[:], 0.0)
            for g in range(NGRP):
                nc.gpsimd.memset(grp[g * QUANT:(g + 1) * QUANT, g:g + 1], 1.0)

            bs = cst_pool.tile([NGRP, NCOLS], f32)

            first_pool_op = [None]

            def do_side(g, V, side, lo, n):
                """Full torsion chain for columns [lo, lo+n) of tile g.
                side: 'd' -> tensor ops on DVE, 'p' -> on Pool."""
                def fld(k):
                    return g[:, k * F + lo:k * F + lo + n]

                def T(tag, d=dt):
                    return tmp_pool.tile([P, n], d, tag=f"{tag}_{side}",
                                         name=f"{tag}_{side}")

                if side == "d":
                    def tt(o, i0, i1, op):
                        nc.vector.tensor_tensor(out=o, in0=i0, in1=i1, op=op)
                else:
                    def tt(o, i0, i1, op):
                        i = nc.gpsimd.tensor_tensor(out=o, in0=i0, in1=i1,
                                                    op=op)
                        if first_pool_op[0] is None:
                            first_pool_op[0] = i
                            add_dep_helper(lib_inst.ins, i.ins, sync=False,
                                           reason="library before pool ops")

                ax, ay, az = fld(0), fld(1), fld(2)
                bx, by, bz = fld(3), fld(4), fld(5)
                cx, cy, cz = fld(6), fld(7), fld(8)

                cs = T("cs")
                def cross1(o, m1, m2, m3, m4):
                    tt(o[:], m1, m2, Alu.mult)
                    tt(cs[:], m3, m4, Alu.mult)
                    tt(o[:], o[:], cs[:], Alu.subtract)

                n1x = T("n1x"); cross1(n1x, ay, bz, az, by)
                n1y = T("n1y"); cross1(n1y, az, bx, ax, bz)
                n1z = T("n1z"); cross1(n1z, ax, by, ay, bx)
                n2x = T("n2x"); cross1(n2x, by, cz, bz, cy)
                n2y = T("n2y"); cross1(n2y, bz, cx, bx, cz)
                n2z = T("n2z"); cross1(n2z, bx, cy, by, cx)

                def dot3(o, u0, u1, u2_, v0, v1, v2):
                    st = T("dt")
                    tt(o[:], u0, v0, Alu.mult)
                    tt(st[:], u1, v1, Alu.mult)
                    tt(o[:], o[:], st[:], Alu.add)
                    tt(st[:], u2_, v2, Alu.mult)
                    tt(o[:], o[:], st[:], Alu.add)

                x = T("x"); dot3(x, n1x[:], n1y[:], n1z[:], n2x[:], n2y[:], n2z[:])
                D = T("D"); dot3(D, ax, ay, az, n2x[:], n2y[:], n2z[:])

                w = T("w"); wt = T("wt")
                nc.scalar.square(w[:], bx)
                nc.scalar.square(wt[:], by)
                tt(w[:], w[:], wt[:], Alu.add)
                nc.scalar.square(wt[:], bz)
                tt(w[:], w[:], wt[:], Alu.add)
                L = T("L"); nc.scalar.sqrt(L[:], w[:])
                Y = T("Y"); tt(Y[:], D[:], L[:], Alu.mult)

                xx = T("xx"); nc.scalar.square(xx[:], x[:])
                YY = T("YY"); nc.scalar.square(YY[:], Y[:])
                q = T("q"); tt(q[:], xx[:], YY[:], Alu.add)
                r = T("r", f32); nc.scalar.sqrt(r[:], q[:])
                uf = T("uf", f32)
                nc.vector.reciprocal_approx_fast(uf[:], r[:])
                u = T("u"); nc.scalar.copy(u[:], uf[:])

                # normalized X = cos(phi), Yh = -sin(phi)
                X = T("X"); tt(X[:], x[:], u[:], Alu.mult)
                Yh = T("Yh"); tt(Yh[:], Y[:], u[:], Alu.mult)
                XX = T("XX"); nc.scalar.square(XX[:], X[:])
                YY1 = T("YY1"); nc.scalar.square(YY1[:], Yh[:])
                c2 = T("c2"); tt(c2[:], XX[:], YY1[:], Alu.subtract)
                s2 = T("s2"); tt(s2[:], X[:], Yh[:], Alu.mult)
                YYd = T("YYd"); nc.scalar.mul(YYd[:], YY1[:], 2.0)
                XXd = T("XXd"); nc.scalar.mul(XXd[:], XX[:], 2.0)
                c3a = T("c3a"); tt(c3a[:], c2[:], YYd[:], Alu.subtract)
                s3a = T("s3a"); tt(s3a[:], c2[:], XXd[:], Alu.add)
                c3 = T("c3"); tt(c3[:], c3a[:], X[:], Alu.mult)
                s3 = T("s3"); tt(s3[:], s3a[:], Yh[:], Alu.mult)

                tB = T("tB")
                Vs = V[:, lo:lo + n]
                # V = A1*X + B1'*Yh + A2*c2 + B2'*s2 + A3*c3 + B3'*s3
                tt(Vs, fld(9), X[:], Alu.mult)
                tt(tB[:], fld(12), Yh[:], Alu.mult)
                tt(Vs, Vs, tB[:], Alu.add)
                tt(tB[:], fld(10), c2[:], Alu.mult)
                tt(Vs, Vs, tB[:], Alu.add)
                tt(tB[:], fld(13), s2[:], Alu.mult)
                tt(Vs, Vs, tB[:], Alu.add)
                tt(tB[:], fld(11), c3[:], Alu.mult)
                tt(Vs, Vs, tB[:], Alu.add)
                tt(tB[:], fld(14), s3[:], Alu.mult)
                tt(Vs, Vs, tB[:], Alu.add)

            def do_tile(t):
                g = gat_pool.tile([P, NPLANES * F], dt, tag="g")
                nc.sync.dma_start(out=g[:], in_=tbl[t * P:(t + 1) * P, :])
                V = tmp_pool.tile([P, F], dt, tag="V", name="V", bufs=2)
                do_side(g[:], V, "d", 0, S)
                if S < F:
                    do_side(g[:], V, "p", S, F - S)
                pt = ps_pool.tile([NGRP, F], f32, tag="ps", name="ps")
                for c0 in range(0, F, 512):
                    c1 = min(c0 + 512, F)
                    nc.tensor.matmul(out=pt[:, c0:c1], lhsT=grp[:],
                                     rhs=V[:, c0:c1], start=True, stop=True)
                nc.scalar.activation(bs[:, t * F:(t + 1) * F], pt[:], Act.Copy)

            def body():
                for t in range(NTILES):
                    do_tile(t)

            if repeat > 1:
                with tc.For_i(0, repeat, 1):
                    body()
            else:
                body()

            nc.sync.dma_start(out=out[:], in_=bs[:])
    nc.compile()
    return nc


# --------------------------------------------------------------------------
# end to end
# --------------------------------------------------------------------------

def prepare(inputs, F=1024, dtype="bf16"):
    pos = np.asarray(inputs["pos"], dtype=np.float32)
    ty = np.asarray(inputs["atom_types"])
    mapping = np.asarray(inputs["mapping"])
    batch = np.asarray(inputs["mapping_batch"]).astype(np.int64)
    base = np.asarray(mapping[0]).astype(np.int64)
    assert all(np.array_equal(np.asarray(mapping[j]), base + j)
               for j in range(1, 4)), "mapping not consecutive; fast path invalid"
    n_win = pos.shape[0] - 3
    fields, Cw = build_window_fields(pos, ty, np.asarray(inputs["thetas"]),
                                     np.asarray(inputs["ks"]))
    streams, blk_labels, NTILES = plan_streams(base, batch, n_win, F)
    np_dtype = ml_dtypes.bfloat16 if dtype == "bf16" else np.float32
    tables = build_core_tables(fields, streams, NTILES, F, np_dtype)
    # angle-independent sum_k term, accumulated host-side
    energy_C = np.bincount(batch, weights=Cw[base].astype(np.float64),
                           minlength=NB)
    plan = dict(NTILES=NTILES, F=F, blk_labels=blk_labels, dtype=dtype,
                energy_C=energy_C)
    return plan, tables


def finish(plan, outs, n_batch=NB):
    """outs: list per core of [NGRP, NCOLS] block sums -> [256] energy."""
    energy = plan["energy_C"].copy()
    for c in range(NCORES):
        bsums = np.asarray(outs[c])          # [NGRP, NCOLS]
        lab = plan["blk_labels"][c]          # [NCOLS*NGRP], -1 = padding
        vals = bsums.T.ravel()               # block (col, grp) order
        m = lab >= 0
        energy += np.bincount(lab[m], weights=vals[m].astype(np.float64),
                              minlength=n_batch)
    return energy.astype(np.float32)


def _kernel_numpy_fallback(pos, atom_types, mapping, mapping_batch, thetas, ks):
    # Correctness safety net for non-consecutive mappings (never expected).
    p0, p1 = pos[mapping[0]], pos[mapping[1]]
    p2, p3 = pos[mapping[2]], pos[mapping[3]]
    dr1, dr2, dr3 = p1 - p0, p2 - p1, p3 - p2
    n1 = np.cross(dr1, dr2); n2 = np.cross(dr2, dr3)
    m1 = np.cross(n1, dr2 / np.linalg.norm(dr2, axis=-1, keepdims=True))
    x = np.sum(n1 * n2, -1); y = np.sum(m1 * n2, -1)
    theta = np.arctan2(y, x)
    t0, t1, t2, t3 = (atom_types[mapping[j]] for j in range(4))
    th = thetas[:, t0, t1, t2, t3]; kk = ks[:, t0, t1, t2, t3]
    degs = np.arange(1, 4)[:, None]
    V = np.sum(kk * (1.0 - np.cos(degs * theta[None, :] - th)), axis=0)
    return np.bincount(mapping_batch, weights=V.astype(np.float64),
                       minlength=256).astype(np.float32)


def kernel(pos, atom_types, mapping, mapping_batch, thetas, ks):
    from concourse.bass_utils import run_bass_kernel_spmd
    pos = np.asarray(pos, dtype=np.float32)
    atom_types = np.asarray(atom_types)
    mapping = np.asarray(mapping)
    mapping_batch = np.asarray(mapping_batch)
    thetas = np.asarray(thetas, dtype=np.float32)
    ks = np.asarray(ks, dtype=np.float32)

    base = np.asarray(mapping[0]).astype(np.int64)
    if not all(np.array_equal(np.asarray(mapping[j]), base + j)
               for j in range(1, 4)):
        print("kernel.py: non-consecutive mapping; numpy fallback",
              file=sys.stderr)
        return _kernel_numpy_fallback(pos, atom_types, mapping, mapping_batch,
                                      thetas, ks)

    inputs = dict(pos=pos, atom_types=atom_types, mapping=mapping,
                  mapping_batch=mapping_batch, thetas=thetas, ks=ks)
    plan, tables = prepare(inputs, F=1024, dtype="bf16")
    nc = build_program(plan["NTILES"], plan["F"], repeat=1,
                       dtype=plan["dtype"])
    in_maps = [{"tbl": tables[c]} for c in range(NCORES)]
    res = run_bass_kernel_spmd(nc, in_maps, list(range(NCORES)))
    outs = [res.results[c]["out"] for c in range(NCORES)]
    return finish(plan, outs).astype(np.float32)


# revision 15
# speedup vs baseline: 13.4389x; 1.1025x over previous
"""Trainium2 Bass kernel for nn_Dihedral (gnn_message_passing, 8 NeuronCores).

kernel(**inputs) -> [256] f32 per-batch dihedral energies.

Design v3 — gather-free streaming, engine-split compute. mapping columns are
consecutive-atom windows (b..b+3), so every per-dihedral quantity except the
batch label is a function of the window start. The host builds, per core, a
batch-sorted per-dihedral stream of 15 bf16 field planes
    [dr1 (3), dr2 (3), dr3 (3), A1..A3, B1'..B3']
(A = -k*cos(th0), B' = sign/2x-folded -k*sin(th0) — the same type-table fold
as v1; the angle-independent C = sum_k term is summed host-side) laid out
tile-contiguously in DRAM, so the device does pure SEQUENTIAL DMA (no
dma_gather, 30B/dihedral vs 256B in v1). Bins are padded to 32-element
blocks along partitions.

Device per core, per [128, F] tile: one dma_start pulls all 15 planes; the
columns are SPLIT between DVE (bf16 2x_1p) and gpsimd, each running the full
torsion chain independently on its slice (no cross-engine dependency
stalls); ACT does squares/sqrts/copies for both slices; PE reduces
32-partition blocks with a [128,4] indicator matmul. The torsion uses the
xy-polynomial form (cos/sin of d*phi expanded in x = n1.n2,
Y = (dr1.n2)|dr2|, u = 1/r — no trig tables). Host: bincount block sums
into 256 bins, sum the 8 per-core partials (the all-reduce of the
sum-sharded output).
"""

import os
import sys
import numpy as np

if "/opt/trn_rl_repo" not in sys.path:
    sys.path.insert(0, "/opt/trn_rl_repo")

import concourse.bass as bass
import concourse.bacc as bacc
import concourse.mybir as mybir
import concourse.tile as tile
from concourse.library_config import standard as std_lib
from concourse.tile_rust import add_dep_helper
import ml_dtypes

P = 128
NCORES = 8
QUANT = 32           # bin padding quantum == PE group size
NGRP = P // QUANT    # 4 partial sums per column
NPLANES = 15
NB = 256


# --------------------------------------------------------------------------
# host-side prep
# --------------------------------------------------------------------------

def build_window_fields(pos, atom_types, thetas, ks):
    """([15, NW] f32 per-window field planes, [NW] f32 C values)."""
    NW = pos.shape[0] - 3
    t3 = thetas.reshape(3, -1).astype(np.float64)
    k3 = ks.reshape(3, -1).astype(np.float64)
    A = (-(k3 * np.cos(t3))).astype(np.float32)      # [3, 390625]
    B = (-(k3 * np.sin(t3))).astype(np.float32)
    C = k3.sum(axis=0).astype(np.float32)
    ty = np.asarray(atom_types).astype(np.int64)
    T4 = ((ty[:NW] * 25 + ty[1:NW + 1]) * 25 + ty[2:NW + 2]) * 25 + ty[3:NW + 3]
    f = np.empty((NPLANES, NW), dtype=np.float32)
    f[0:3] = (pos[1:NW + 1] - pos[0:NW]).T
    f[3:6] = (pos[2:NW + 2] - pos[1:NW + 1]).T
    f[6:9] = (pos[3:NW + 3] - pos[2:NW + 2]).T
    f[9] = A[0, T4]; f[10] = A[1, T4]; f[11] = A[2, T4]
    f[12] = -B[0, T4]; f[13] = -2.0 * B[1, T4]; f[14] = -B[2, T4]
    return f, C[T4]


def plan_streams(base, batch, n_win, F):
    """Per-core batch-sorted window-index streams, bins padded to QUANT,
    common NTILES across cores."""
    SUBW = (n_win + NCORES - 1) // NCORES
    core_of = base // SUBW

    idx_streams = []
    lab_streams = []
    for c in range(NCORES):
        sel = np.nonzero(core_of == c)[0]        # batch-sorted already
        lab = batch[sel]
        cnt = np.bincount(lab, minlength=NB)
        pcnt = ((cnt + QUANT - 1) // QUANT) * QUANT
        total = int(pcnt.sum())
        idx_out = np.full(total, -1, dtype=np.int64)
        ends = np.cumsum(pcnt)
        starts = ends - pcnt
        within = np.arange(len(sel)) - np.repeat(np.cumsum(cnt) - cnt, cnt)
        idx_out[starts[lab] + within] = base[sel]
        blk_lab = np.repeat(np.arange(NB, dtype=np.int32), pcnt // QUANT)
        idx_streams.append(idx_out)
        lab_streams.append(blk_lab)

    max_cols = max((len(s) + P - 1) // P for s in idx_streams)
    NTILES = (max_cols + F - 1) // F
    NCOLS = NTILES * F

    streams = np.full((NCORES, NCOLS * P), -1, dtype=np.int64)
    blk_labels = np.full((NCORES, NCOLS * NGRP), -1, dtype=np.int32)
    for c in range(NCORES):
        streams[c, :len(idx_streams[c])] = idx_streams[c]
        blk_labels[c, :len(lab_streams[c])] = lab_streams[c]
    return streams, blk_labels, NTILES


def build_core_tables(fields, streams, NTILES, F, np_dtype):
    """[NCORES, NTILES*P, NPLANES*F] plane tables (tile-contiguous DRAM)."""
    tables = np.empty((NCORES, NTILES * P, NPLANES * F), dtype=np_dtype)
    for c in range(NCORES):
        w = streams[c]
        dummy = w < 0
        vals = fields[:, np.where(dummy, 0, w)]          # [15, NCOLS*P]
        if dummy.any():
            vals[9:15][:, dummy] = 0.0                   # V = 0 for padding
        # stream s = t*(P*F) + col*P + p  ->  dram[t*P + p, k*F + col]
        v4 = vals.reshape(NPLANES, NTILES, F, P)
        tables[c] = (v4.transpose(1, 3, 0, 2)
                     .reshape(NTILES * P, NPLANES * F).astype(np_dtype))
    return tables


# --------------------------------------------------------------------------
# device program
# --------------------------------------------------------------------------

def build_program(NTILES, F, repeat=1, dtype="bf16", split=None,
                  gat_bufs=None, tmp_bufs=None):
    if split is None:
        split = float(os.environ.get("DK_SPLIT", "1.0"))
    if gat_bufs is None:
        gat_bufs = int(os.environ.get("DK_GATB", "2"))
    if tmp_bufs is None:
        tmp_bufs = int(os.environ.get("DK_TMPB", "1"))
    f32 = mybir.dt.float32
    dt = mybir.dt.bfloat16 if dtype == "bf16" else f32
    Alu = mybir.AluOpType
    Act = mybir.ActivationFunctionType
    NCOLS = NTILES * F
    S = (int(F * split) // 4) * 4            # DVE columns; rest go to Pool

    nc = bacc.Bacc("TRN2", target_bir_lowering=False, debug=False)
    tbl = nc.dram_tensor("tbl", [NTILES * P, NPLANES * F], dt,
                         kind="ExternalInput").ap()
    out = nc.dram_tensor("out", [NGRP, NCOLS], f32, kind="ExternalOutput").ap()

    with tile.TileContext(nc) as tc:
        with (
            tc.tile_pool(name="gat", bufs=gat_bufs) as gat_pool,
            tc.tile_pool(name="tmp", bufs=tmp_bufs) as tmp_pool,
            tc.tile_pool(name="cst", bufs=1) as cst_pool,
            tc.tile_pool(name="ps", bufs=2, space="PSUM") as ps_pool,
        ):
            lib_inst = nc.gpsimd.load_library(std_lib)

            grp = cst_pool.tile([P, NGRP], dt)
            nc.gpsimd.memset(grp[:], 0.0)
            for g in range(NGRP):
                nc.gpsimd.memset(grp[g * QUANT:(g + 1) * QUANT, g:g + 1], 1.0)

            bs = cst_pool.tile([NGRP, NCOLS], f32)

            def do_side(g, side, lo, n):
                """Full torsion chain for columns [lo, lo+n) of tile g;
                tensor ops on DVE, squares/sqrt/affine on ACT."""
                def fld(k):
                    return g[:, k * F + lo:k * F + lo + n]

                def T(tag, d=dt):
                    return tmp_pool.tile([P, n], d, tag=f"{tag}_{side}",
                                         name=f"{tag}_{side}")

                def tt(o, i0, i1, op):
                    nc.vector.tensor_tensor(out=o, in0=i0, in1=i1, op=op)

                ax, ay, az = fld(0), fld(1), fld(2)
                bx, by, bz = fld(3), fld(4), fld(5)
                cx, cy, cz = fld(6), fld(7), fld(8)

                cs = T("cs")
                def cross1(o, m1, m2, m3, m4):
                    tt(o[:], m1, m2, Alu.mult)
                    tt(cs[:], m3, m4, Alu.mult)
                    tt(o[:], o[:], cs[:], Alu.subtract)

                n1x = T("n1x"); cross1(n1x, ay, bz, az, by)
                n1y = T("n1y"); cross1(n1y, az, bx, ax, bz)
                n1z = T("n1z"); cross1(n1z, ax, by, ay, bx)
                n2x = T("n2x"); cross1(n2x, by, cz, bz, cy)
                n2y = T("n2y"); cross1(n2y, bz, cx, bx, cz)
                n2z = T("n2z"); cross1(n2z, bx, cy, by, cx)

                def dot3(o, u0, u1, u2_, v0, v1, v2):
                    st = T("dt")
                    tt(o[:], u0, v0, Alu.mult)
                    tt(st[:], u1, v1, Alu.mult)
                    tt(o[:], o[:], st[:], Alu.add)
                    tt(st[:], u2_, v2, Alu.mult)
                    tt(o[:], o[:], st[:], Alu.add)

                x = T("x"); dot3(x, n1x[:], n1y[:], n1z[:], n2x[:], n2y[:], n2z[:])
                D = T("D"); dot3(D, ax, ay, az, n2x[:], n2y[:], n2z[:])

                w = T("w"); wt = T("wt")
                nc.scalar.square(w[:], bx)
                nc.scalar.square(wt[:], by)
                tt(w[:], w[:], wt[:], Alu.add)
                nc.scalar.square(wt[:], bz)
                tt(w[:], w[:], wt[:], Alu.add)
                L = T("L"); nc.scalar.sqrt(L[:], w[:])
                Y = T("Y"); tt(Y[:], D[:], L[:], Alu.mult)

                xx = T("xx"); nc.scalar.square(xx[:], x[:])
                YY = T("YY"); nc.scalar.square(YY[:], Y[:])
                q = T("q"); tt(q[:], xx[:], YY[:], Alu.add)
                r = T("r", f32); nc.scalar.sqrt(r[:], q[:])
                uf = T("uf", f32)
                nc.vector.reciprocal_approx_fast(uf[:], r[:])
                u = T("u"); nc.scalar.copy(u[:], uf[:])

                # normalized X = cos(phi), Yh = -sin(phi); X^2 + Yh^2 = 1
                Act_ = mybir.ActivationFunctionType
                X = T("X"); tt(X[:], x[:], u[:], Alu.mult)
                Yh = T("Yh"); tt(Yh[:], Y[:], u[:], Alu.mult)
                YY1 = T("YY1"); nc.scalar.square(YY1[:], Yh[:])
                c2 = T("c2")
                nc.scalar.activation(c2[:], YY1[:], Act_.Copy, bias=1.0,
                                     scale=-2.0)
                c3a = T("c3a")
                nc.scalar.activation(c3a[:], YY1[:], Act_.Copy, bias=1.0,
                                     scale=-4.0)
                s3a = T("s3a")
                nc.scalar.activation(s3a[:], YY1[:], Act_.Copy, bias=3.0,
                                     scale=-4.0)
                s2 = T("s2"); tt(s2[:], X[:], Yh[:], Alu.mult)
                c3 = T("c3"); tt(c3[:], c3a[:], X[:], Alu.mult)
                s3 = T("s3"); tt(s3[:], s3a[:], Yh[:], Alu.mult)

                # six Fourier terms; their sum is absorbed into the PSUM
                # accumulation of the block-sum matmuls
                terms = []
                for i, (coef, trig) in enumerate([
                        (9, X), (12, Yh), (10, c2), (13, s2),
                        (11, c3), (14, s3)]):
                    tm = tmp_pool.tile([P, n], dt, tag=f"tm{i}_{side}",
                                       name=f"tm{i}_{side}")
                    tt(tm[:], fld(coef), trig[:], Alu.mult)
                    terms.append(tm)
                return terms

            def do_tile(t):
                g = gat_pool.tile([P, NPLANES * F], dt, tag="g")
                nc.sync.dma_start(out=g[:], in_=tbl[t * P:(t + 1) * P, :])
                terms = do_side(g[:], "d", 0, F)
                pt = ps_pool.tile([NGRP, F], f32, tag="ps", name="ps")
                for c0 in range(0, F, 512):
                    c1 = min(c0 + 512, F)
                    for i, tm in enumerate(terms):
                        nc.tensor.matmul(out=pt[:, c0:c1], lhsT=grp[:],
                                         rhs=tm[:, c0:c1],
                                         start=(i == 0), stop=(i == 5))
                nc.scalar.activation(bs[:, t * F:(t + 1) * F], pt[:], Act.Copy)

            def body():
                for t in range(NTILES):
                    do_tile(t)

            if repeat > 1:
                with tc.For_i(0, repeat, 1):
                    body()
            else:
                body()

            nc.sync.dma_start(out=out[:], in_=bs[:])
    nc.compile()
    return nc


# --------------------------------------------------------------------------
# end to end
# --------------------------------------------------------------------------

def prepare(inputs, F=1024, dtype="bf16"):
    pos = np.asarray(inputs["pos"], dtype=np.float32)
    ty = np.asarray(inputs["atom_types"])
    mapping = np.asarray(inputs["mapping"])
    batch = np.asarray(inputs["mapping_batch"]).astype(np.int64)
    base = np.asarray(mapping[0]).astype(np.int64)
    assert all(np.array_equal(np.asarray(mapping[j]), base + j)
               for j in range(1, 4)), "mapping not consecutive; fast path invalid"
    n_win = pos.shape[0] - 3
    fields, Cw = build_window_fields(pos, ty, np.asarray(inputs["thetas"]),
                                     np.asarray(inputs["ks"]))
    streams, blk_labels, NTILES = plan_streams(base, batch, n_win, F)
    np_dtype = ml_dtypes.bfloat16 if dtype == "bf16" else np.float32
    tables = build_core_tables(fields, streams, NTILES, F, np_dtype)
    # angle-independent sum_k term, accumulated host-side
    energy_C = np.bincount(batch, weights=Cw[base].astype(np.float64),
                           minlength=NB)
    plan = dict(NTILES=NTILES, F=F, blk_labels=blk_labels, dtype=dtype,
                energy_C=energy_C)
    return plan, tables


def finish(plan, outs, n_batch=NB):
    """outs: list per core of [NGRP, NCOLS] block sums -> [256] energy."""
    energy = plan["energy_C"].copy()
    for c in range(NCORES):
        bsums = np.asarray(outs[c])          # [NGRP, NCOLS]
        lab = plan["blk_labels"][c]          # [NCOLS*NGRP], -1 = padding
        vals = bsums.T.ravel()               # block (col, grp) order
        m = lab >= 0
        energy += np.bincount(lab[m], weights=vals[m].astype(np.float64),
                              minlength=n_batch)
    return energy.astype(np.float32)


def _kernel_numpy_fallback(pos, atom_types, mapping, mapping_batch, thetas, ks):
    # Correctness safety net for non-consecutive mappings (never expected).
    p0, p1 = pos[mapping[0]], pos[mapping[1]]
    p2, p3 = pos[mapping[2]], pos[mapping[3]]
    dr1, dr2, dr3 = p1 - p0, p2 - p1, p3 - p2
    n1 = np.cross(dr1, dr2); n2 = np.cross(dr2, dr3)
    m1 = np.cross(n1, dr2 / np.linalg.norm(dr2, axis=-1, keepdims=True))
    x = np.sum(n1 * n2, -1); y = np.sum(m1 * n2, -1)
    theta = np.arctan2(y, x)
    t0, t1, t2, t3 = (atom_types[mapping[j]] for j in range(4))
    th = thetas[:, t0, t1, t2, t3]; kk = ks[:, t0, t1, t2, t3]
    degs = np.arange(1, 4)[:, None]
    V = np.sum(kk * (1.0 - np.cos(degs * theta[None, :] - th)), axis=0)
    return np.bincount(mapping_batch, weights=V.astype(np.float64),
                       minlength=256).astype(np.float32)


def kernel(pos, atom_types, mapping, mapping_batch, thetas, ks):
    from concourse.bass_utils import run_bass_kernel_spmd
    pos = np.asarray(pos, dtype=np.float32)
    atom_types = np.asarray(atom_types)
    mapping = np.asarray(mapping)
    mapping_batch = np.asarray(mapping_batch)
    thetas = np.asarray(thetas, dtype=np.float32)
    ks = np.asarray(ks, dtype=np.float32)

    base = np.asarray(mapping[0]).astype(np.int64)
    if not all(np.array_equal(np.asarray(mapping[j]), base + j)
               for j in range(1, 4)):
        print("kernel.py: non-consecutive mapping; numpy fallback",
              file=sys.stderr)
        return _kernel_numpy_fallback(pos, atom_types, mapping, mapping_batch,
                                      thetas, ks)

    inputs = dict(pos=pos, atom_types=atom_types, mapping=mapping,
                  mapping_batch=mapping_batch, thetas=thetas, ks=ks)
    plan, tables = prepare(inputs, F=1024, dtype="bf16")
    nc = build_program(plan["NTILES"], plan["F"], repeat=1,
                       dtype=plan["dtype"])
    in_maps = [{"tbl": tables[c]} for c in range(NCORES)]
    res = run_bass_kernel_spmd(nc, in_maps, list(range(NCORES)))
    outs = [res.results[c]["out"] for c in range(NCORES)]
    return finish(plan, outs).astype(np.float32)
